# revision 2
# baseline (speedup 1.0000x reference)
"""Delta-threshold encoder (DeltaModulator) Trainium2 Bass kernel.

Input  x: (2048, 128, 320) f32.
Output y: (2048, 128, 620) f32 = [UP flags (300) | DN flags (300) | tail (20)].

Per (batch,row) element the reference runs a 300-step serial scan:
    up_t = x_t > dc + d;  dn_t = x_t < dc - d;  dc <- x_t if (up|dn) else dc

Strategy (8 NeuronCores, batch-sharded, no communication):
  - 32768 elements per core, laid out as 4 groups x 128 partitions x 64
    elements, each element's 320-float row contiguous in DRAM.
  - The serial recurrence runs as ONE custom DVE instruction per time step
    over (128, 64) elements:  dc_t = select((x>dc+d)|(x<dc-d), x, dc),
    writing the dc trace to SBUF time slots.
  - Key identity (exact in fp32): a trigger always changes dc, so
        up_t == (dc_t > dc_{t-1})   and   dn_t == (dc_t < dc_{t-1}).
    The UP/DN flag planes are therefore bulk tensor_tensor comparisons over
    the dc trace -- no per-step comparison instructions.
  - UP overwrites the (dead) x chunk buffer; DN overwrites the dc trace
    in-place (writes trail reads in the streaming pipeline), so SBUF holds
    only two double-buffered streams. Time is chunked 2x150 so every DMA
    moves >=600B contiguous runs. The 20-float tail is a DRAM->DRAM copy.
"""

import numpy as np

import concourse.bacc as bacc
import concourse.tile as tile
from concourse import mybir, dve_ops
from concourse.dve_spec import Spec, Src0, Src1, C0, C1, select, lower, _has_src1
from concourse.dve_uop import DveOpSpec
from concourse.bass_utils import run_bass_kernel_spmd

DELTA = 0.02
B, R, TIN = 2048, 128, 320
TSCAN, TTAIL = 300, 20
TOUT = TSCAN * 2 + TTAIL  # 620
NCORES = 8
EPC = B * R // NCORES     # elements per core = 32768
G, P, F = 4, 128, 64      # groups x partitions x elems-per-partition = EPC
TC = 150                  # time-chunk length (2 chunks cover the 300 scanned cols)
NCHUNK = TSCAN // TC


def _delta_step_op():
    """Register (once) the fused scan-step DVE op:
    out = select((in0 > in1 + s0) | (in0 < in1 + s1), in0, in1)."""
    name = "DELTA_STEP_ANT"
    for op in dve_ops.OPS:
        if op.name == name:
            return op
    up = Src0 > (Src1 + C0)
    dn = Src0 < (Src1 + C1)
    spec = Spec(
        body=select(up | dn, Src0, Src1),
        reference=lambda in0, in1, s0, s1, imm2: np.where(
            (in0 > in1 + s0) | (in0 < in1 + s1), in0, in1
        ).astype(np.float32),
    )
    row = dve_ops._CUSTOM_DVE_ROW_BASE + len(dve_ops.OPS)
    dve_ops._SUB_OPCODE_FOR_NAME[name] = row
    shas = {
        v: DveOpSpec(
            name=name, opcode=row, uops=lower(spec, ver=v), rd1_en=_has_src1(spec)
        ).sha(v)
        for v in ("v3", "v4")
    }
    op = dve_ops.DveOp(name, spec, subdim=False, uops_sha=shas)
    dve_ops.OPS.append(op)
    dve_ops.CUSTOM_DVE_SPECS[name] = spec
    return op


def _build_module():
    step_op = _delta_step_op()
    nc = bacc.Bacc(
        "TRN2",
        target_bir_lowering=False,
        debug=False,
        enable_asserts=False,
        num_devices=NCORES,
    )
    x = nc.dram_tensor("x", [G, P, F, TIN], mybir.dt.float32, kind="ExternalInput")
    y = nc.dram_tensor("y", [G, P, F, TOUT], mybir.dt.float32, kind="ExternalOutput")

    is_gt = mybir.AluOpType.is_gt
    is_lt = mybir.AluOpType.is_lt

    with tile.TileContext(nc) as tc:
        with (
            tc.tile_pool(name="xbuf", bufs=2) as xpool,
            tc.tile_pool(name="dcbuf", bufs=2) as dcpool,
        ):
            for g in range(G):
                prev_dc = None
                for c in range(NCHUNK):
                    t0 = c * TC
                    xt = xpool.tile([P, F, TC], mybir.dt.float32, tag="x")
                    nc.sync.dma_start(xt[:], x[g, :, :, t0 : t0 + TC])
                    dc = dcpool.tile([P, F, TC + 1], mybir.dt.float32, tag="dc")
                    # slot 0 = dc entering this chunk
                    if c == 0:
                        nc.gpsimd.memset(dc[:, :, 0], 0.0)
                    else:
                        nc.vector.tensor_copy(dc[:, :, 0], prev_dc[:, :, TC])
                    # serial scan: one fused DVE op per time step
                    for tau in range(TC):
                        nc.vector._custom_dve(
                            step_op,
                            out=dc[:, :, tau + 1],
                            in0=xt[:, :, tau],
                            in1=dc[:, :, tau],
                            s0=DELTA,
                            s1=-DELTA,
                        )
                    # bulk flags from the dc trace (exact):
                    # diff = dc_next - dc_prev, in-place (writes trail reads). Pool.
                    nc.gpsimd.tensor_tensor(
                        dc[:, :, 0:TC],
                        dc[:, :, 1 : TC + 1],
                        dc[:, :, 0:TC],
                        mybir.AluOpType.subtract,
                    )
                    # UP = (diff > 0) -> overwrites the dead x chunk. DVE.
                    nc.vector.tensor_scalar(
                        xt[:], dc[:, :, 0:TC], 0.0, None, is_gt
                    )
                    # DN = relu(sign(-diff)), in-place over diff. ACT.
                    nc.scalar.activation(
                        dc[:, :, 0:TC],
                        dc[:, :, 0:TC],
                        mybir.ActivationFunctionType.Sign,
                        scale=-1.0,
                    )
                    nc.scalar.activation(
                        dc[:, :, 0:TC],
                        dc[:, :, 0:TC],
                        mybir.ActivationFunctionType.Relu,
                    )
                    nc.sync.dma_start(y[g, :, :, t0 : t0 + TC], xt[:])
                    nc.sync.dma_start(
                        y[g, :, :, TSCAN + t0 : TSCAN + t0 + TC], dc[:, :, 0:TC]
                    )
                    prev_dc = dc
                # tail passthrough, DRAM->DRAM
                nc.sync.dma_start(
                    y[g, :, :, 2 * TSCAN : TOUT], x[g, :, :, TSCAN:TIN]
                )
    nc.compile()
    return nc


_NC_CACHE = []


def _get_module():
    if not _NC_CACHE:
        _NC_CACHE.append(_build_module())
    return _NC_CACHE[0]


def kernel(x: np.ndarray) -> np.ndarray:
    x = np.ascontiguousarray(np.asarray(x, dtype=np.float32))
    assert x.shape == (B, R, TIN)
    nc = _get_module()
    shards = x.reshape(NCORES, G, P, F, TIN)
    in_maps = [{"x": shards[c]} for c in range(NCORES)]
    res = run_bass_kernel_spmd(nc, in_maps, core_ids=list(range(NCORES)))
    y = np.stack([res.results[c]["y"] for c in range(NCORES)], axis=0)
    return y.reshape(B, R, TOUT)


if __name__ == "__main__":
    rng = np.random.default_rng(0)
    xs = rng.standard_normal((B, R, TIN)).astype(np.float32)
    out = kernel(xs)
    print(out.shape, out.dtype)


# revision 5
# speedup vs baseline: 34.2053x; 34.2053x over previous
"""Delta-threshold encoder (DeltaModulator) Trainium2 Bass kernel.

Input  x: (2048, 128, 320) f32.
Output y: (2048, 128, 620) f32 = [UP flags (300) | DN flags (300) | tail (20)].

Per (batch,row) element the reference runs a 300-step serial scan:
    up_t = x_t > dc + d;  dn_t = x_t < dc - d;  dc <- x_t if (up|dn) else dc

Strategy (8 NeuronCores, batch-sharded, no communication):
  - 32768 elements per core, laid out as 4 groups x 128 partitions x 64
    elements, each element's 320-float row contiguous in DRAM.
  - The serial recurrence runs as ONE custom DVE instruction per time step
    over (128, 64) elements:  dc_t = select((x>dc+d)|(x<dc-d), x, dc),
    writing the dc trace into SBUF time slots. DVE does nothing else.
  - Key identity (exact in fp32): a trigger always changes dc, so
        up_t == (dc_t > dc_{t-1})   and   dn_t == (dc_t < dc_{t-1}).
    The flag planes are bulk ops over the dc trace:
        diff = dc_next - dc_prev   (Pool, in-place, writes trail reads)
        UP   = diff > 0            (Pool tensor_scalar, into the dead x chunk)
        DN   = relu(sign(-diff))   (ACT, in-place over diff)
  - Time is chunked [0,150) + [150,320): every DMA moves >=600B contiguous
    runs per element. The 20-float tail rides the second x chunk in and is
    appended to the DN chunk so DN+tail is one 680B-per-element stream out.
"""

import numpy as np

import concourse.bacc as bacc
import concourse.tile as tile
from concourse import mybir, dve_ops
from concourse.dve_spec import Spec, Src0, Src1, C0, C1, select, lower, _has_src1
from concourse.dve_uop import DveOpSpec
from concourse.bass_utils import run_bass_kernel_spmd

DELTA = 0.02
B, R, TIN = 2048, 128, 320
TSCAN, TTAIL = 300, 20
TOUT = TSCAN * 2 + TTAIL  # 620
NCORES = 8
EPC = B * R // NCORES     # elements per core = 32768
G, P, F = 4, 128, 64      # groups x partitions x elems-per-partition = EPC
TC = 150                  # chunk A scan length; chunk B covers [150,320)
XB = TIN - TC             # 170 cols in x chunk B (scan cols + tail)


def _delta_step_op():
    """Register (once) the fused scan-step DVE op:
    out = select((in0 > in1 + s0) | (in0 < in1 + s1), in0, in1)."""
    name = "DELTA_STEP_ANT"
    for op in dve_ops.OPS:
        if op.name == name:
            return op
    up = Src0 > (Src1 + C0)
    dn = Src0 < (Src1 + C1)
    spec = Spec(
        body=select(up | dn, Src0, Src1),
        reference=lambda in0, in1, s0, s1, imm2: np.where(
            (in0 > in1 + s0) | (in0 < in1 + s1), in0, in1
        ).astype(np.float32),
    )
    row = dve_ops._CUSTOM_DVE_ROW_BASE + len(dve_ops.OPS)
    dve_ops._SUB_OPCODE_FOR_NAME[name] = row
    shas = {
        v: DveOpSpec(
            name=name, opcode=row, uops=lower(spec, ver=v), rd1_en=_has_src1(spec)
        ).sha(v)
        for v in ("v3", "v4")
    }
    op = dve_ops.DveOp(name, spec, subdim=False, uops_sha=shas)
    dve_ops.OPS.append(op)
    dve_ops.CUSTOM_DVE_SPECS[name] = spec
    return op


def _build_module():
    step_op = _delta_step_op()
    nc = bacc.Bacc(
        "TRN2",
        target_bir_lowering=False,
        debug=False,
        enable_asserts=False,
        num_devices=NCORES,
    )
    x = nc.dram_tensor("x", [G, P, F, TIN], mybir.dt.float32, kind="ExternalInput")
    y = nc.dram_tensor("y", [G, P, F, TOUT], mybir.dt.float32, kind="ExternalOutput")

    is_gt = mybir.AluOpType.is_gt
    sub = mybir.AluOpType.subtract
    Sign = mybir.ActivationFunctionType.Sign
    Relu = mybir.ActivationFunctionType.Relu

    with tile.TileContext(nc) as tc:
        with (
            tc.tile_pool(name="xbuf", bufs=2) as xpool,
            tc.tile_pool(name="dcbuf", bufs=2) as dcpool,
        ):
            def scan_chunk(xt, dt, n_steps):
                for tau in range(n_steps):
                    nc.vector._custom_dve(
                        step_op,
                        out=dt[:, :, tau + 1],
                        in0=xt[:, :, tau],
                        in1=dt[:, :, tau],
                        s0=DELTA,
                        s1=-DELTA,
                    )

            def flags_chunk(xt, dt):
                # diff = dc_next - dc_prev, in-place (writes trail reads)
                nc.gpsimd.tensor_tensor(
                    dt[:, :, 0:TC], dt[:, :, 1 : TC + 1], dt[:, :, 0:TC], sub
                )
                # UP = diff > 0 -> dead region of the x chunk (DVE TS, 2x mode)
                nc.vector.tensor_scalar(
                    xt[:, :, 0:TC], dt[:, :, 0:TC], 0.0, None, is_gt
                )
                # DN = relu(sign(-diff)), in-place
                nc.scalar.activation(dt[:, :, 0:TC], dt[:, :, 0:TC], Sign, scale=-1.0)
                nc.scalar.activation(dt[:, :, 0:TC], dt[:, :, 0:TC], Relu)

            for g in range(G):
                # ---- chunk A: scan cols [0,150) ----
                xa = xpool.tile([P, F, XB], mybir.dt.float32, tag="x")
                nc.sync.dma_start(xa[:, :, 0:TC], x[g, :, :, 0:TC])
                xb = xpool.tile([P, F, XB], mybir.dt.float32, tag="x")
                nc.sync.dma_start(xb[:], x[g, :, :, TC:TIN])
                da = dcpool.tile([P, F, XB], mybir.dt.float32, tag="dc")
                nc.gpsimd.memset(da[:, :, 0], 0.0)  # dc before step 0
                scan_chunk(xa, da, TC)
                # ---- chunk B: scan cols [150,300); x cols [150,320) ----
                db = dcpool.tile([P, F, XB], mybir.dt.float32, tag="dc")
                nc.scalar.copy(db[:, :, 0], da[:, :, TC])  # carry dc across chunks
                scan_chunk(xb, db, TC)
                flags_chunk(xa, da)
                nc.sync.dma_start(y[g, :, :, 0:TC], xa[:, :, 0:TC])
                nc.scalar.dma_start(y[g, :, :, TSCAN : TSCAN + TC], da[:, :, 0:TC])
                flags_chunk(xb, db)
                # tail: append x[300:320) after DN in chunk B's dc buffer so
                # DN+tail leaves as one 680B-per-element stream
                nc.scalar.copy(db[:, :, TC:XB], xb[:, :, TC:XB])
                nc.sync.dma_start(y[g, :, :, TC:TSCAN], xb[:, :, 0:TC])
                nc.scalar.dma_start(y[g, :, :, TSCAN + TC : TOUT], db[:, :, 0:XB])
    nc.compile()
    return nc


_NC_CACHE = []


def _get_module():
    if not _NC_CACHE:
        _NC_CACHE.append(_build_module())
    return _NC_CACHE[0]


def kernel(x: np.ndarray) -> np.ndarray:
    x = np.ascontiguousarray(np.asarray(x, dtype=np.float32))
    assert x.shape == (B, R, TIN)
    nc = _get_module()
    shards = x.reshape(NCORES, G, P, F, TIN)
    in_maps = [{"x": shards[c]} for c in range(NCORES)]
    res = run_bass_kernel_spmd(nc, in_maps, core_ids=list(range(NCORES)))
    y = np.stack([res.results[c]["y"] for c in range(NCORES)], axis=0)
    return y.reshape(B, R, TOUT)


if __name__ == "__main__":
    rng = np.random.default_rng(0)
    xs = rng.standard_normal((B, R, TIN)).astype(np.float32)
    out = kernel(xs)
    print(out.shape, out.dtype)


# revision 8
# speedup vs baseline: 38.3841x; 1.1222x over previous
"""Delta-threshold encoder (DeltaModulator) Trainium2 Bass kernel.

Input  x: (2048, 128, 320) f32.
Output y: (2048, 128, 620) f32 = [UP flags (300) | DN flags (300) | tail (20)].

Per (batch,row) element the reference runs a 300-step serial scan:
    up_t = x_t > dc + d;  dn_t = x_t < dc - d;  dc <- x_t if (up|dn) else dc

Strategy (8 NeuronCores, batch-sharded, no communication):
  - 32768 elements per core, laid out as 4 groups x 128 partitions x 64
    elements, each element's 320-float row contiguous in DRAM.
  - The serial recurrence runs as ONE custom DVE instruction per time step
    over (128, 64) elements:  dc_t = select((x>dc+d)|(x<dc-d), x, dc),
    writing the dc trace into SBUF time slots. DVE does nothing else.
  - Key identity (exact in fp32): a trigger always changes dc, so
        up_t == (dc_t > dc_{t-1})   and   dn_t == (dc_t < dc_{t-1}).
    The flag planes are bulk ops over the dc trace:
        diff = dc_next - dc_prev   (Pool, in-place, writes trail reads)
        UP   = diff > 0            (Pool tensor_scalar, into the dead x chunk)
        DN   = relu(sign(-diff))   (ACT, in-place over diff)
  - Time is chunked [0,150) + [150,320): every DMA moves >=600B contiguous
    runs per element. The 20-float tail rides the second x chunk in and is
    appended to the DN chunk so DN+tail is one 680B-per-element stream out.
"""

import numpy as np

import concourse.bacc as bacc
import concourse.tile as tile
from concourse import mybir, dve_ops
from concourse.dve_spec import Spec, Src0, Src1, C0, C1, select, lower, _has_src1
from concourse.dve_uop import DveOpSpec
from concourse.bass_utils import run_bass_kernel_spmd

DELTA = 0.02
B, R, TIN = 2048, 128, 320
TSCAN, TTAIL = 300, 20
TOUT = TSCAN * 2 + TTAIL  # 620
NCORES = 8
EPC = B * R // NCORES     # elements per core = 32768
G, P, F = 4, 128, 64      # groups x partitions x elems-per-partition = EPC
TC = 150                  # chunk A scan length; chunk B covers [150,320)
XB = TIN - TC             # 170 cols in x chunk B (scan cols + tail)


def _delta_step_op():
    """Register (once) the fused scan-step DVE op:
    out = select((in0 > in1 + s0) | (in0 < in1 + s1), in0, in1)."""
    name = "DELTA_STEP_ANT"
    for op in dve_ops.OPS:
        if op.name == name:
            return op
    up = Src0 > (Src1 + C0)
    dn = Src0 < (Src1 + C1)
    spec = Spec(
        body=select(up | dn, Src0, Src1),
        reference=lambda in0, in1, s0, s1, imm2: np.where(
            (in0 > in1 + s0) | (in0 < in1 + s1), in0, in1
        ).astype(np.float32),
    )
    row = dve_ops._CUSTOM_DVE_ROW_BASE + len(dve_ops.OPS)
    dve_ops._SUB_OPCODE_FOR_NAME[name] = row
    shas = {
        v: DveOpSpec(
            name=name, opcode=row, uops=lower(spec, ver=v), rd1_en=_has_src1(spec)
        ).sha(v)
        for v in ("v3", "v4")
    }
    op = dve_ops.DveOp(name, spec, subdim=False, uops_sha=shas)
    dve_ops.OPS.append(op)
    dve_ops.CUSTOM_DVE_SPECS[name] = spec
    return op


def _build_module():
    step_op = _delta_step_op()
    nc = bacc.Bacc(
        "TRN2",
        target_bir_lowering=False,
        debug=False,
        enable_asserts=False,
        num_devices=NCORES,
    )
    x = nc.dram_tensor("x", [G, P, F, TIN], mybir.dt.float32, kind="ExternalInput")
    y = nc.dram_tensor("y", [G, P, F, TOUT], mybir.dt.float32, kind="ExternalOutput")

    is_gt = mybir.AluOpType.is_gt
    sub = mybir.AluOpType.subtract
    Sign = mybir.ActivationFunctionType.Sign
    Relu = mybir.ActivationFunctionType.Relu

    with tile.TileContext(nc) as tc:
        with (
            tc.tile_pool(name="xbuf", bufs=2) as xpool,
            tc.tile_pool(name="dcbuf", bufs=2) as dcpool,
        ):
            def scan_chunk(xt, dt, n_steps):
                for tau in range(n_steps):
                    nc.vector._custom_dve(
                        step_op,
                        out=dt[:, :, tau + 1],
                        in0=xt[:, :, tau],
                        in1=dt[:, :, tau],
                        s0=DELTA,
                        s1=-DELTA,
                    )

            def flags_chunk(xt, dt):
                # diff = dc_next - dc_prev, in-place (writes trail reads)
                nc.gpsimd.tensor_tensor(
                    dt[:, :, 0:TC], dt[:, :, 1 : TC + 1], dt[:, :, 0:TC], sub
                )
                # UP = relu(sign(diff)) -> dead region of the x chunk (ACT)
                nc.scalar.activation(xt[:, :, 0:TC], dt[:, :, 0:TC], Sign)
                nc.scalar.activation(xt[:, :, 0:TC], xt[:, :, 0:TC], Relu)
                # DN = relu(sign(-diff)), in-place (ACT)
                nc.scalar.activation(dt[:, :, 0:TC], dt[:, :, 0:TC], Sign, scale=-1.0)
                nc.scalar.activation(dt[:, :, 0:TC], dt[:, :, 0:TC], Relu)

            for g in range(G):
                # ---- chunk A: scan cols [0,150) ----
                xa = xpool.tile([P, F, XB], mybir.dt.float32, tag="x")
                nc.sync.dma_start(xa[:, :, 0:TC], x[g, :, :, 0:TC])
                xb = xpool.tile([P, F, XB], mybir.dt.float32, tag="x")
                nc.sync.dma_start(xb[:], x[g, :, :, TC:TIN])
                da = dcpool.tile([P, F, XB], mybir.dt.float32, tag="dc")
                nc.gpsimd.memset(da[:, :, 0], 0.0)  # dc before step 0
                scan_chunk(xa, da, TC)
                # ---- chunk B: scan cols [150,300); x cols [150,320) ----
                db = dcpool.tile([P, F, XB], mybir.dt.float32, tag="dc")
                nc.gpsimd.tensor_copy(db[:, :, 0], da[:, :, TC])  # carry dc
                scan_chunk(xb, db, TC)
                flags_chunk(xa, da)
                nc.sync.dma_start(y[g, :, :, 0:TC], xa[:, :, 0:TC])
                nc.scalar.dma_start(y[g, :, :, TSCAN : TSCAN + TC], da[:, :, 0:TC])
                flags_chunk(xb, db)
                # tail: append x[300:320) after DN in chunk B's dc buffer so
                # DN+tail leaves as one 680B-per-element stream
                nc.gpsimd.tensor_copy(db[:, :, TC:XB], xb[:, :, TC:XB])
                nc.sync.dma_start(y[g, :, :, TC:TSCAN], xb[:, :, 0:TC])
                nc.scalar.dma_start(y[g, :, :, TSCAN + TC : TOUT], db[:, :, 0:XB])
    nc.compile()
    return nc


_NC_CACHE = []


def _get_module():
    if not _NC_CACHE:
        _NC_CACHE.append(_build_module())
    return _NC_CACHE[0]


def kernel(x: np.ndarray) -> np.ndarray:
    x = np.ascontiguousarray(np.asarray(x, dtype=np.float32))
    assert x.shape == (B, R, TIN)
    nc = _get_module()
    shards = x.reshape(NCORES, G, P, F, TIN)
    in_maps = [{"x": shards[c]} for c in range(NCORES)]
    res = run_bass_kernel_spmd(nc, in_maps, core_ids=list(range(NCORES)))
    y = np.stack([res.results[c]["y"] for c in range(NCORES)], axis=0)
    return y.reshape(B, R, TOUT)


if __name__ == "__main__":
    rng = np.random.default_rng(0)
    xs = rng.standard_normal((B, R, TIN)).astype(np.float32)
    out = kernel(xs)
    print(out.shape, out.dtype)
